# revision 1
# baseline (speedup 1.0000x reference)
"""Trainium2 Bass kernel for the dense_transformer problem.

Data-parallel over batch: 8 NeuronCores x (B/8) sequences each.
All heavy matmuls run in bf16 operands with fp32 PSUM accumulation.

Self-contained: only imports numpy + installed concourse package.
"""

import numpy as np
from contextlib import ExitStack

import concourse.bass as bass
import concourse.bacc as bacc
import concourse.mybir as mybir
import concourse.tile as tile
from concourse.bass_utils import run_bass_kernel_spmd
from concourse.masks import make_identity, make_upper_triangular

# problem dims (hardcoded per harness contract)
B, L, D, C, NQ, KW, NL = 64, 1024, 256, 256, 10000, 4, 3
NCORES = 8
P = 128
F32 = mybir.dt.float32
BF16 = mybir.dt.bfloat16
I32 = mybir.dt.int32
AF = mybir.ActivationFunctionType
ALU = mybir.AluOpType

LT = L // P           # 8 token tiles of 128
NKT = (4 * D + C) // P  # 10 feature tiles of H
PAD = KW - 1          # 3 causal pad cols


def _emit(nc, tc, ctx, dram, nb, repeat=1):
    sb = ctx.enter_context(tc.tile_pool(name="sb", bufs=1))
    seq = ctx.enter_context(tc.tile_pool(name="seq", bufs=1))
    wk = ctx.enter_context(tc.tile_pool(name="wk", bufs=1))
    ps = ctx.enter_context(tc.tile_pool(name="ps", bufs=1, space="PSUM"))

    # ---------------- constants ----------------
    ident16 = sb.tile([P, P], BF16, tag="ident16")
    make_identity(nc, ident16[:])
    ident32 = sb.tile([P, P], F32, tag="ident32")
    make_identity(nc, ident32[:])
    smask = sb.tile([P, P], BF16, tag="smask")  # strict upper: 1.0 where part < free
    make_upper_triangular(nc, smask[:], val=1.0, diag=False)
    ones16 = sb.tile([P, 1], BF16, tag="ones16")
    nc.gpsimd.memset(ones16[:], 1.0)

    # biases (fp32, per-partition layout)
    w1b = sb.tile([P, 2], F32, tag="w1b")
    w2b = sb.tile([P, 2], F32, tag="w2b")
    for dh in range(2):
        nc.sync.dma_start(out=w1b[:, dh : dh + 1], in_=dram["w1b"][dh * P : (dh + 1) * P, None])
        nc.sync.dma_start(out=w2b[:, dh : dh + 1], in_=dram["w2b"][dh * P : (dh + 1) * P, None])
    convb = sb.tile([P, NL * 4], F32, tag="convb")
    for ly in range(NL):
        for oc in range(4):
            nc.sync.dma_start(
                out=convb[:, ly * 4 + oc : ly * 4 + oc + 1],
                in_=dram["convb"][ly, oc * P : (oc + 1) * P, None],
            )

    # Ec rows flat on partition 0: [ec0 | ec1 | diff] bf16
    ones_row = sb.tile([1, L], BF16, tag="ones_row")
    nc.gpsimd.memset(ones_row[:], 1.0)
    ec_st = wk.tile([1, 2 * D], F32, tag="ec_st")
    nc.sync.dma_start(out=ec_st[:], in_=dram["ec"].rearrange("a b -> (a b)")[None, :])
    ec16f = sb.tile([1, 3 * D], BF16, tag="ec16f")
    nc.vector.tensor_copy(ec16f[0:1, 0 : 2 * D], ec_st[:])
    nc.vector.tensor_sub(ec16f[0:1, 2 * D : 3 * D], ec16f[0:1, D : 2 * D], ec16f[0:1, 0:D])

    # ---------------- weight prep (emitted later for overlap) ----------------
    w1t = sb.tile([P, NKT * D], BF16, tag="w1t")
    w2t = sb.tile([P, NKT * D], BF16, tag="w2t")
    cw = sb.tile([P, NL * KW * 2 * 2 * D], BF16, tag="cw")

    def emit_weights():
        # W1T/W2T: [1280, 256] bf16 as [128, 2560]; block kt at cols kt*256, half dh at +dh*128
        for name, wt in (("w1w", w1t), ("w2w", w2t)):
            for dh in range(2):
                stg = wk.tile([P, NKT * P], F32, tag="wstage", bufs=2, name=f"wstg_{name}{dh}")
                nc.sync.dma_start(out=stg[:], in_=dram[name][dh * P : (dh + 1) * P, :])
                for kt in range(NKT):
                    tp = ps.tile([P, P], F32, tag="small", bufs=3, name=f"wtp_{name}{dh}_{kt}")
                    nc.tensor.transpose(out=tp[:], in_=stg[:, kt * P : (kt + 1) * P], identity=ident32[:])
                    nc.vector.tensor_copy(wt[:, kt * D + dh * P : kt * D + (dh + 1) * P], tp[:])
        # conv weights bf16: [(ly,k,cin) -> [128, 512]] at cols ((ly*4+k)*2+cin)*512
        for ly in range(NL):
            for k in range(KW):
                for cin in range(2):
                    stg = wk.tile([P, 2 * D], F32, tag="cwstage", bufs=3, name=f"cwstg{ly}_{k}_{cin}")
                    nc.sync.dma_start(out=stg[:], in_=dram["convw"][ly, k, cin * P : (cin + 1) * P, :])
                    base = ((ly * KW + k) * 2 + cin) * 2 * D
                    nc.vector.tensor_copy(cw[:, base : base + 2 * D], stg[:])

    # ---------------- per-sequence pipeline stages ----------------
    issued = {}

    def prep_issue(bg):
        """DMA-only: start gathers + staging loads for sequence bg early."""
        qraw = seq.tile([P, LT * D], F32, tag="qraw", bufs=2, name=f"qraw{bg}")
        for lt in range(LT):
            idx = wk.tile([P, 1], I32, tag="idx", bufs=4, name=f"idx{bg}_{lt}")
            nc.sync.dma_start(out=idx[:], in_=dram["qseq"][bg, lt * P : (lt + 1) * P, None])
            nc.gpsimd.indirect_dma_start(
                out=qraw[:, lt * D : (lt + 1) * D], out_offset=None, in_=dram["eq"][:],
                in_offset=bass.IndirectOffsetOnAxis(ap=idx[:, :1], axis=0),
            )
        cqcs = []
        for ct in range(2):
            stg = wk.tile([P, L], F32, tag="cqcstage", bufs=3, name=f"cqcstg{bg}_{ct}")
            nc.sync.dma_start(out=stg[:], in_=dram["cqct"][bg, ct * P : (ct + 1) * P, :])
            cqcs.append(stg)
        corr_i = wk.tile([1, L], I32, tag="corr_i", bufs=2, name=f"corri{bg}")
        nc.sync.dma_start(out=corr_i[:], in_=dram["cseq"][bg : bg + 1, :])
        issued[bg] = (qraw, cqcs, corr_i)

    def prep(bg):
        """Consume staged data: build LIS + HT feature blocks for sequence bg."""
        qraw, cqcs, corr_i = issued.pop(bg)
        # LIS [l, 512] bf16 as [128, 8*512]; qe at lt*512, ce at lt*512+256
        lis = seq.tile([P, LT * 2 * D], BF16, tag="lis", bufs=2, name=f"lis{bg}")
        # HT [1280, 1024] bf16 as [128, 10*1024]; kt 0-1 qeT, 2-3 ceT, 4-7 hrpT, 8-9 cqcT
        ht = seq.tile([P, NKT * L], BF16, tag="ht", bufs=2, name=f"ht{bg}")

        for ct in range(2):
            nc.vector.tensor_copy(ht[:, (8 + ct) * L : (9 + ct) * L], cqcs[ct][:])
        for lt in range(LT):
            nc.vector.tensor_copy(lis[:, lt * 2 * D : lt * 2 * D + D], qraw[:, lt * D : (lt + 1) * D])
        corr_row = wk.tile([1, L], BF16, tag="corr_row", bufs=2, name=f"corrr{bg}")
        nc.vector.tensor_copy(corr_row[:], corr_i[:])

        # ce into LIS: ce = c (x) diff + 1 (x) ec0 via two K=1 matmuls
        for lt in range(LT):
            cep = ps.tile([P, D], F32, tag="mm", bufs=5, name=f"cep{bg}_{lt}")
            nc.tensor.matmul(
                cep[:], lhsT=corr_row[0:1, lt * P : (lt + 1) * P],
                rhs=ec16f[0:1, 2 * D : 3 * D], start=True, stop=False,
            )
            nc.tensor.matmul(
                cep[:], lhsT=ones_row[0:1, lt * P : (lt + 1) * P],
                rhs=ec16f[0:1, 0:D], start=False, stop=True,
            )
            nc.vector.tensor_copy(lis[:, lt * 2 * D + D : (lt + 1) * 2 * D], cep[:])

        # ceT into HT
        for dh in range(2):
            for lt2 in range(2):
                cetp = ps.tile([P, 4 * P], F32, tag="mm", bufs=5, name=f"cetp{bg}_{dh}_{lt2}")
                nc.tensor.matmul(
                    cetp[:], lhsT=ec16f[0:1, 2 * D + dh * P : 2 * D + (dh + 1) * P],
                    rhs=corr_row[0:1, lt2 * 4 * P : (lt2 + 1) * 4 * P],
                    start=True, stop=False,
                )
                nc.tensor.matmul(
                    cetp[:], lhsT=ec16f[0:1, dh * P : (dh + 1) * P],
                    rhs=ones_row[0:1, lt2 * 4 * P : (lt2 + 1) * 4 * P],
                    start=False, stop=True,
                )
                nc.vector.tensor_copy(
                    ht[:, (2 + dh) * L + lt2 * 4 * P : (2 + dh) * L + (lt2 + 1) * 4 * P], cetp[:]
                )

        # qeT into HT via PE transpose of LIS qe cols
        for lt in range(LT):
            for dh in range(2):
                tp = ps.tile([P, P], BF16, tag="small", bufs=3, name=f"qtp{bg}_{lt}_{dh}")
                nc.tensor.transpose(
                    out=tp[:], in_=lis[:, lt * 2 * D + dh * P : lt * 2 * D + (dh + 1) * P],
                    identity=ident16[:],
                )
                nc.vector.tensor_copy(ht[:, dh * L + lt * P : dh * L + (lt + 1) * P], tp[:])
        return lis, ht

    def attn(bg, lis, ht):
        """Wide score tiles [j, i-half] then per-i-block HRP into HT."""
        for iw in range(2):
            jmax = iw * 4 + 4
            twides = []
            for jb in range(jmax):
                # cols below the diagonal block are never read by the hrp/s
                # loops (jb <= ib), so compute only [rel:] of the wide tile
                rel = max(jb * P - iw * 4 * P, 0)
                n_live = 4 * P - rel
                scp = ps.tile([P, 4 * P], F32, tag="mm", bufs=5, name=f"scp{bg}_{iw}_{jb}")
                for kd in range(2):
                    nc.tensor.matmul(
                        scp[:, 0:n_live],
                        lhsT=ht[:, kd * L + jb * P : kd * L + (jb + 1) * P],
                        rhs=ht[:, kd * L + iw * 4 * P + rel : kd * L + (iw + 1) * 4 * P],
                        start=(kd == 0), stop=(kd == 1),
                    )
                tw = wk.tile([P, 4 * P], BF16, tag="T", bufs=13, name=f"tw{bg}_{iw}_{jb}")
                nc.scalar.activation(tw[:, rel : 4 * P], scp[:, 0:n_live], AF.Exp)
                if jb * P - iw * 4 * P >= 0:
                    nc.vector.tensor_mul(tw[:, rel : rel + P], tw[:, rel : rel + P], smask[:])
                twides.append(tw)
            for isub in range(4):
                ib = iw * 4 + isub
                hrp_ps = ps.tile([P, 2 * D], F32, tag="mm", bufs=5, name=f"hrpp{bg}_{ib}")
                s_ps = ps.tile([P, 1], F32, tag="small", bufs=3, name=f"sps{bg}_{ib}")
                for jb in range(ib + 1):
                    tsl = twides[jb][:, isub * P : (isub + 1) * P]
                    nc.tensor.matmul(
                        hrp_ps[:], lhsT=tsl, rhs=lis[:, jb * 2 * D : (jb + 1) * 2 * D],
                        start=(jb == 0), stop=(jb == ib),
                    )
                    nc.tensor.matmul(
                        s_ps[:], lhsT=tsl, rhs=ones16[:],
                        start=(jb == 0), stop=(jb == ib),
                    )
                sp = wk.tile([P, 1], F32, tag="sp", bufs=2, name=f"sp{bg}_{ib}")
                nc.vector.tensor_scalar_add(sp[:], s_ps[:], 1e-8)
                nc.vector.reciprocal(sp[:], sp[:])
                hrp16 = wk.tile([P, 2 * D], BF16, tag="hrp16", bufs=3, name=f"hrp16{bg}_{ib}")
                nc.scalar.activation(hrp16[:], hrp_ps[:], AF.Identity, scale=sp[:, 0:1])
                for dh in range(4):
                    tp = ps.tile([P, P], BF16, tag="small", bufs=3, name=f"htp{bg}_{ib}_{dh}")
                    nc.tensor.transpose(
                        out=tp[:], in_=hrp16[:, dh * P : (dh + 1) * P], identity=ident16[:]
                    )
                    nc.vector.tensor_copy(ht[:, (4 + dh) * L + ib * P : (4 + dh) * L + (ib + 1) * P], tp[:])

    def tail(bg, lis, ht):
        """MLP + conv stack + predict + output DMA."""
        xcur = [seq.tile([P, PAD + L], BF16, tag="xbuf", bufs=6, name=f"xq{bg}_{h}") for h in range(2)]
        for h in range(2):
            nc.vector.memset(xcur[h][:, 0:PAD], 0.0)
        for lt2 in range(2):
            for dh in range(2):
                p1 = ps.tile([P, 4 * P], F32, tag="mm", bufs=5, name=f"p1_{bg}_{lt2}_{dh}")
                p2 = ps.tile([P, 4 * P], F32, tag="mm", bufs=5, name=f"p2_{bg}_{lt2}_{dh}")
                for kt in range(NKT):
                    nc.tensor.matmul(
                        p1[:], lhsT=w1t[:, kt * D + dh * P : kt * D + (dh + 1) * P],
                        rhs=ht[:, kt * L + lt2 * 4 * P : kt * L + (lt2 + 1) * 4 * P],
                        start=(kt == 0), stop=(kt == NKT - 1),
                    )
                for kt in range(NKT):
                    nc.tensor.matmul(
                        p2[:], lhsT=w2t[:, kt * D + dh * P : kt * D + (dh + 1) * P],
                        rhs=ht[:, kt * L + lt2 * 4 * P : kt * L + (lt2 + 1) * 4 * P],
                        start=(kt == 0), stop=(kt == NKT - 1),
                    )
                gate = wk.tile([P, 4 * P], BF16, tag="gate", bufs=6, name=f"gmlp{bg}_{lt2}_{dh}")
                nc.scalar.activation(gate[:], p2[:], AF.Sigmoid, bias=w2b[:, dh : dh + 1])
                nc.vector.scalar_tensor_tensor(
                    out=xcur[dh][:, PAD + lt2 * 4 * P : PAD + (lt2 + 1) * 4 * P],
                    in0=p1[:], scalar=w1b[:, dh : dh + 1], in1=gate[:],
                    op0=ALU.add, op1=ALU.mult,
                )

        for ly in range(NL):
            xnext = [seq.tile([P, PAD + L], BF16, tag="xbuf", bufs=6, name=f"xn{bg}_{ly}_{h}") for h in range(2)]
            for h in range(2):
                nc.vector.memset(xnext[h][:, 0:PAD], 0.0)
            for lt2 in range(2):
                for pair in range(2):
                    oc_a, oc_b = pair, 2 + pair
                    pb = ps.tile([P, 4 * P], F32, tag="mm", bufs=5, name=f"pb{bg}_{ly}_{lt2}_{pair}")
                    for k in range(KW):
                        for cin in range(2):
                            base = ((ly * KW + k) * 2 + cin) * 2 * D
                            nc.tensor.matmul(
                                pb[:], lhsT=cw[:, base + oc_b * P : base + (oc_b + 1) * P],
                                rhs=xcur[cin][:, lt2 * 4 * P + k : lt2 * 4 * P + k + 4 * P],
                                start=(k == 0 and cin == 0), stop=(k == KW - 1 and cin == 1),
                            )
                    gate = wk.tile([P, 4 * P], BF16, tag="gate", bufs=6, name=f"gcv{bg}_{ly}_{lt2}_{pair}")
                    nc.scalar.activation(gate[:], pb[:], AF.Sigmoid, bias=convb[:, ly * 4 + oc_b : ly * 4 + oc_b + 1])
                    pa = ps.tile([P, 4 * P], F32, tag="mm", bufs=5, name=f"pa{bg}_{ly}_{lt2}_{pair}")
                    for k in range(KW):
                        for cin in range(2):
                            base = ((ly * KW + k) * 2 + cin) * 2 * D
                            nc.tensor.matmul(
                                pa[:], lhsT=cw[:, base + oc_a * P : base + (oc_a + 1) * P],
                                rhs=xcur[cin][:, lt2 * 4 * P + k : lt2 * 4 * P + k + 4 * P],
                                start=(k == 0 and cin == 0), stop=(k == KW - 1 and cin == 1),
                            )
                    glu = wk.tile([P, 4 * P], BF16, tag="glu", bufs=3, name=f"glu{bg}_{ly}_{lt2}_{pair}")
                    nc.vector.scalar_tensor_tensor(
                        out=glu[:], in0=pa[:], scalar=convb[:, ly * 4 + oc_a : ly * 4 + oc_a + 1],
                        in1=gate[:], op0=ALU.add, op1=ALU.mult,
                    )
                    nc.vector.tensor_add(
                        xnext[pair][:, PAD + lt2 * 4 * P : PAD + (lt2 + 1) * 4 * P],
                        glu[:],
                        xcur[pair][:, PAD + lt2 * 4 * P : PAD + (lt2 + 1) * 4 * P],
                    )
            xcur = xnext

        prods = []
        for cin in range(2):
            prod = wk.tile([P, L - 1], BF16, tag="prod", bufs=2, name=f"prod{bg}_{cin}")
            nc.vector.tensor_mul(
                prod[:], xcur[cin][:, PAD : PAD + L - 1], ht[:, cin * L + 1 : cin * L + L]
            )
            prods.append(prod)
        osb = wk.tile([1, L - 1], F32, tag="osb", bufs=2, name=f"osb{bg}")
        for half in range(2):
            n = 4 * P if half == 0 else L - 1 - 4 * P
            zp = ps.tile([1, 4 * P], F32, tag="small", bufs=3, name=f"zp{bg}_{half}")
            for cin in range(2):
                nc.tensor.matmul(
                    zp[:, :n], lhsT=ones16[:], rhs=prods[cin][:, half * 4 * P : half * 4 * P + n],
                    start=(cin == 0), stop=(cin == 1),
                )
            nc.scalar.activation(osb[:, half * 4 * P : half * 4 * P + n], zp[:, :n], AF.Sigmoid)
        nc.sync.dma_start(out=dram["out"][bg : bg + 1, :], in_=osb[:])

    # ---------------- emission schedule (software-pipelined) ----------------
    def pipeline():
        for bg in range(nb):
            prep_issue(bg)
            st = prep(bg)
            attn(bg, *st)
            if bg == 0:
                emit_weights()
            tail(bg, *st)

    if repeat > 1:
        emit_weights()
        loop_cm = tc.For_i(0, repeat, 1)
        loop_cm.__enter__()
        import os as _os
        _sched = _os.environ.get("SCHED", "seq")
        if _sched == "seq":
            for bg in range(nb):
                prep_issue(bg)
                st = prep(bg)
                attn(bg, *st)
                tail(bg, *st)
        else:
            prep_issue(0)
            states = {0: prep(0)}
            for bg in range(nb):
                if bg + 1 < nb:
                    prep_issue(bg + 1)
                attn(bg, *states[bg])
                tail(bg, *states.pop(bg))
                if bg + 1 < nb:
                    states[bg + 1] = prep(bg + 1)
        loop_cm.__exit__(None, None, None)
    else:
        pipeline()



def build(nb, repeat=1):
    nc = bacc.Bacc("TRN2", target_bir_lowering=False, debug=False)
    dram = {
        "qseq": nc.dram_tensor("qseq", [nb, L], I32, kind="ExternalInput").ap(),
        "cseq": nc.dram_tensor("cseq", [nb, L], I32, kind="ExternalInput").ap(),
        "cqct": nc.dram_tensor("cqct", [nb, C, L], F32, kind="ExternalInput").ap(),
        "eq": nc.dram_tensor("eq", [NQ, D], F32, kind="ExternalInput").ap(),
        "ec": nc.dram_tensor("ec", [2, D], F32, kind="ExternalInput").ap(),
        "w1w": nc.dram_tensor("w1w", [D, 4 * D + C], F32, kind="ExternalInput").ap(),
        "w1b": nc.dram_tensor("w1b", [D], F32, kind="ExternalInput").ap(),
        "w2w": nc.dram_tensor("w2w", [D, 4 * D + C], F32, kind="ExternalInput").ap(),
        "w2b": nc.dram_tensor("w2b", [D], F32, kind="ExternalInput").ap(),
        "convw": nc.dram_tensor("convw", [NL, KW, D, 2 * D], F32, kind="ExternalInput").ap(),
        "convb": nc.dram_tensor("convb", [NL, 2 * D], F32, kind="ExternalInput").ap(),
        "out": nc.dram_tensor("out", [nb, L - 1], F32, kind="ExternalOutput").ap(),
    }
    with tile.TileContext(nc) as tc:
        with ExitStack() as ctx:
            _emit(nc, tc, ctx, dram, nb, repeat)
    nc.compile()
    return nc


_built = {}


def make_in_maps(inputs, nb):
    inp = {k: np.asarray(v) for k, v in inputs.items()}
    qseq = np.ascontiguousarray(inp["question_seq"].astype(np.int32))
    cseq = np.ascontiguousarray(inp["correctness_seq"].astype(np.int32))
    cqct = np.ascontiguousarray(
        np.transpose(inp["cqc_seq"].astype(np.float32), (0, 2, 1))
    )
    base = {
        "eq": np.ascontiguousarray(inp["Eq"].astype(np.float32)),
        "ec": np.ascontiguousarray(inp["Ec"].astype(np.float32)),
        "w1w": np.ascontiguousarray(inp["W1_w"].astype(np.float32)),
        "w1b": np.ascontiguousarray(inp["W1_b"].astype(np.float32)),
        "w2w": np.ascontiguousarray(inp["W2_w"].astype(np.float32)),
        "w2b": np.ascontiguousarray(inp["W2_b"].astype(np.float32)),
        "convw": np.ascontiguousarray(inp["conv_w"].astype(np.float32)),
        "convb": np.ascontiguousarray(inp["conv_b"].astype(np.float32)),
    }
    in_maps = []
    for cid in range(NCORES):
        sl = slice(cid * nb, (cid + 1) * nb)
        m = dict(base)
        m["qseq"] = qseq[sl]
        m["cseq"] = cseq[sl]
        m["cqct"] = cqct[sl]
        in_maps.append(m)
    return in_maps


def run_sharded(inputs, nb=B // NCORES, trace=False, **kw):
    if nb not in _built:
        _built[nb] = build(nb)
    nc = _built[nb]
    in_maps = make_in_maps(inputs, nb)
    res = run_bass_kernel_spmd(nc, in_maps, list(range(NCORES)), trace=trace, **kw)
    out = np.concatenate([res.results[c]["out"] for c in range(NCORES)], axis=0)
    return out.astype(np.float32), res


def kernel(**inputs):
    out, _ = run_sharded(inputs)
    return out

